# revision 24
# baseline (speedup 1.0000x reference)
"""Single-head causal self-attention on 8 Trainium2 NeuronCores (Bass/Tile).

Problem: x [1024, 256, 384], Wq/Wk/Wv [384, 64] ->
  q,k,v = x@W;  wei = softmax(mask(q k^T / sqrt(384)));  out = wei @ v
Output: [1024, 256, 64] fp32.

Strategy (data-parallel over batch, 128 batches per core):
  - Host pre-transposes x to bf16 xt4[g, p, c, j] = x[4g + j//256, j%256,
    128c+p] (groups of 4 batches = 2 pairs): contraction dim C=384 on SBUF
    partitions, one 6KB-per-partition DMA per group.
  - Per batch pair:
      ps_qk [128,512] = [Wk|Wq]^T x^T          (3 mm, N=512, bf16)
      kq slot [128,640] fp8e4 <- ONE DVE cast from PSUM (k rows 0:64,
        q rows 64:128); cols 512:640 hold persistent zeros.
      v-proj reuses x-stationary form (12 mm, N=64, bf16) -> ps_v [t,h];
        one gpsimd copy -> vaug [128,2,2,66] bf16 w/ ones at col 64.
      psw [128,1024] = wei^T blocks via fp8 DoubleRow matmuls: the second
        k-tile is the zero pad (cols 512:640), so every wei matmul runs at
        0.5 cycles/row (4 mm per pair).
      P [128,2,384] bf16 = exp(psw/sqrt(384))   (ONE ACT op per pair)
      mask diag blocks via 4D strided AP        (ONE DVE mul per pair)
      pso [128,4,65]: out[t,h]+denom: lhsT=P-block stationary, rhs=vaug
        (3 mm per batch, N=65, bf16; col 64 = softmax denominator)
      out_sb bf16 <- gpsimd copy of pso (numerator + denominator shipped;
        the divide happens on host in fp32).
  - Three-stage software pipeline (front_a p | front_b p-1 | back p-2).
  - Output [g, p, 8, 65] bf16 blocks; host divides and reassembles [b,t,h].
"""

from contextlib import ExitStack

import numpy as np
import ml_dtypes

import concourse.bass as bass
import concourse.bacc as bacc
import concourse.tile as tile
from concourse import mybir
from concourse.bass_utils import run_bass_kernel_spmd

N_CORES = 8
B = 1024
T = 256
C = 384
H = 64
BPC = B // N_CORES  # 128 batches per core
NCHUNK = C // 128  # 3
NGROUP = BPC // 4  # 32 groups (2 pairs) per core
SCALE = float(C) ** -0.5

F32 = mybir.dt.float32
BF16 = mybir.dt.bfloat16
F8 = mybir.dt.float8e4
BF = ml_dtypes.bfloat16


def build_nc(bpc: int = BPC):
    npair = bpc // 2
    ngroup = bpc // 4
    nc = bacc.Bacc(
        "TRN2", target_bir_lowering=False, debug=False, num_devices=N_CORES
    )

    xt4 = nc.dram_tensor("xt4", [ngroup, 128, NCHUNK, 1024], BF16, kind="ExternalInput").ap()
    wkq = nc.dram_tensor("wkq", [128, NCHUNK, 128], BF16, kind="ExternalInput").ap()
    wv = nc.dram_tensor("wv", [128, NCHUNK, H], BF16, kind="ExternalInput").ap()
    mask = nc.dram_tensor("mask", [128, 128], BF16, kind="ExternalInput").ap()
    outF = nc.dram_tensor("outF", [ngroup, 128, 8, H + 1], BF16, kind="ExternalOutput").ap()

    with ExitStack() as ctx:
        tc = ctx.enter_context(tile.TileContext(nc))

        const = ctx.enter_context(tc.tile_pool(name="const", bufs=1))
        xt_pool = ctx.enter_context(tc.tile_pool(name="xt", bufs=3))
        wkq_sb = const.tile([128, NCHUNK, 128], BF16, tag="wkq")
        nc.sync.dma_start(wkq_sb[:], wkq)
        wv_sb = const.tile([128, NCHUNK, H], BF16, tag="wv")
        nc.sync.dma_start(wv_sb[:], wv)
        # Cached fill register for the gpsimd affine-select causal mask.
        fill_reg = nc.gpsimd.to_reg(0.0)

        NSLOT = 4
        # Persistent kq slots [128, slot, 5, 128] fp8: k^T rows 0:64 /
        # q^T rows 64:128 in blocks 0:4; block 4 stays zero (the DoubleRow
        # zero k-tile). One tile so the per-group q-relocation DMA can cover
        # two adjacent slots with a single strided access pattern.
        kqs = const.tile([128, NSLOT, 5, 128], F8, tag="kqs")
        nc.gpsimd.memset(kqs[:, :, 4, :], 0.0)
        # q relocated to base-partition 0 (matmul fmap must share the weight
        # tile's start partition): qs[0:64, slot, t] <- kqs[64:128, slot, t].
        qs = const.tile([64, NSLOT, 512], F8, tag="qs")
        # Persistent v_aug pair-tiles [batch, s-half, 66]: v at [., j, i, 0:64],
        # ones at col 64 (softmax denominator trick). 6 slots span the
        # 4-stage pipeline distance between the v copy and the PV matmuls.
        NSLOT_V = 6
        vaug = []
        for i in range(NSLOT_V):
            v_t = const.tile([128, 2, 2, 66], BF16, tag=f"vaug{i}")
            nc.gpsimd.memset(v_t[:, :, :, 64:65], 1.0)
            vaug.append(v_t)

        p_pool = ctx.enter_context(tc.tile_pool(name="pp", bufs=5))
        o_pool = ctx.enter_context(tc.tile_pool(name="oo", bufs=2))
        psqk_pool = ctx.enter_context(tc.tile_pool(name="psqk", bufs=2, space="PSUM"))
        psv_pool = ctx.enter_context(tc.tile_pool(name="psv", bufs=2, space="PSUM"))
        psw_pool = ctx.enter_context(tc.tile_pool(name="psw", bufs=3, space="PSUM"))
        pso_pool = ctx.enter_context(tc.tile_pool(name="pso", bufs=1, space="PSUM"))

        xt_tiles = {}
        st_a = {}  # pair -> (kq slot, vaug slot)
        st_b = {}  # pair -> (P, kq slot? not needed, vaug slot)
        st_o = {}  # group -> out_sb

        def front_a(p):
            """DMA + qk projection + fp8 cast + v projection for pair p."""
            g, r = divmod(p, 2)
            if r == 0:
                xt = xt_pool.tile([128, NCHUNK, 1024], BF16, tag="xt")
                nc.sync.dma_start(xt[:], xt4[g])
                xt_tiles[g] = xt
            xt = xt_tiles[g]
            base = 512 * r

            ps_qk = psqk_pool.tile([128, 512], F32, tag="psqk")
            for c in range(NCHUNK):
                nc.tensor.matmul(
                    ps_qk[:],
                    lhsT=wkq_sb[:, c, :],
                    rhs=xt[:, c, base : base + 512],
                    start=(c == 0),
                    stop=(c == NCHUNK - 1),
                )
            s = p % NSLOT
            nc.vector.tensor_copy(
                kqs[:, s, 0:4, :].rearrange("p a b -> p (a b)"), ps_qk[:]
            )

            ps_v = psv_pool.tile([128, 4, H], F32, tag="psv")
            for blk in range(4):
                for c in range(NCHUNK):
                    nc.tensor.matmul(
                        ps_v[:, blk, :],
                        lhsT=xt[:, c, base + 128 * blk : base + 128 * (blk + 1)],
                        rhs=wv_sb[:, c, :],
                        start=(c == 0),
                        stop=(c == NCHUNK - 1),
                    )
            vslot = vaug[p % NSLOT_V]
            nc.scalar.copy(vslot[:, 0, :, 0:64], ps_v[:, 0:2, :])
            nc.vector.tensor_copy(vslot[:, 1, :, 0:64], ps_v[:, 2:4, :])
            st_a[p] = (s, vslot)

        def qmove(g):
            """Relocate both pairs' q halves to partition base 0 (one DMA)."""
            sA = (2 * g) % NSLOT
            nc.sync.dma_start(
                qs[:, sA : sA + 2, :],
                kqs[64:128, sA : sA + 2, 0:4, :].rearrange("p a b c -> p a (b c)"),
            )

        def front_b(p):
            """wei DoubleRow matmuls + exp + mask for pair p, per batch so
            the wei->exp->wei loop-carried chain interleaves across the two
            one-bank psw buffers instead of serializing pair-to-pair."""
            s, vslot = st_a.pop(p)

            P = p_pool.tile([128, 2, 3, 128], BF16, tag="p")
            for j in range(2):
                psw = psw_pool.tile([128, 512], F32, tag="psw")
                # fp8 DoubleRow: k-tile 0 is the real k block (block i of the
                # slot), k-tile 1 is the persistent zero pad at block index 4
                # -- step-slice i::(4-i) selects blocks {i, 4}.
                # (s-block 0) x (t 0:256) -> cols 0:256
                nc.tensor.matmul(
                    psw[:, 0:256],
                    lhsT=kqs[0:64, s, 2 * j :: 4 - 2 * j, :],
                    rhs=qs[:, s, 256 * j : 256 * j + 256]
                    .unsqueeze(1)
                    .broadcast_to([64, 2, 256]),
                    start=True,
                    stop=True,
                    perf_mode=mybir.MatmulPerfMode.DoubleRow,
                )
                # (s-block 1) x (t 128:256) -> cols 256:384
                nc.tensor.matmul(
                    psw[:, 256:384],
                    lhsT=kqs[0:64, s, 2 * j + 1 :: 3 - 2 * j, :],
                    rhs=qs[:, s, 256 * j + 128 : 256 * j + 256]
                    .unsqueeze(1)
                    .broadcast_to([64, 2, 128]),
                    start=True,
                    stop=True,
                    perf_mode=mybir.MatmulPerfMode.DoubleRow,
                )
                # Blocks land as [s0t0 | s0t1 | s1t1] = P[:, j, {0,1,2}, :].
                nc.scalar.activation(
                    P[:, j, :, :].rearrange("p a b -> p (a b)"),
                    psw[:, 0:384],
                    mybir.ActivationFunctionType.Exp,
                    scale=SCALE,
                )
                # Causal mask on the diag blocks (0 and 2) via GPSIMD affine
                # select: keep P[s, t] where t - s >= 0, else fill 0.
                nc.gpsimd.affine_select(
                    out=P[:, j, 0::2, :],
                    in_=P[:, j, 0::2, :],
                    pattern=[[0, 2], [1, 128]],
                    compare_op=mybir.AluOpType.is_ge,
                    fill=fill_reg,
                    base=0,
                    channel_multiplier=-1,
                )
            st_b[p] = (P, vslot)

        def back(p):
            """Out matmuls + output copy + output DMA for pair p."""
            g, r = divmod(p, 2)
            P, vslot = st_b.pop(p)
            pso_full = pso_pool.tile([128, 4, 2 * H], F32, tag="pso")
            pso = pso_full[:, :, 0 : H + 1]
            for j in range(2):
                nc.tensor.matmul(
                    pso[:, 2 * j, :],
                    lhsT=P[:, j, 0, :],
                    rhs=vslot[:, j, 0, 0:65],
                    start=True,
                    stop=True,
                )
                nc.tensor.matmul(
                    pso[:, 2 * j + 1, :],
                    lhsT=P[:, j, 1, :],
                    rhs=vslot[:, j, 0, 0:65],
                    start=True,
                    stop=False,
                )
                nc.tensor.matmul(
                    pso[:, 2 * j + 1, :],
                    lhsT=P[:, j, 2, :],
                    rhs=vslot[:, j, 1, 0:65],
                    start=False,
                    stop=True,
                )

            if r == 0:
                out_sb = o_pool.tile([128, 8, H + 1], BF16, tag="out")
                st_o[g] = out_sb
            else:
                out_sb = st_o[g]
            nc.vector.tensor_copy(out_sb[:, 4 * r : 4 * r + 4, :], pso)
            if r == 1:
                nc.sync.dma_start(outF[g], st_o.pop(g)[:])

        # Software pipeline: fa(p) | back(p-5) | fb(p-3), with the q
        # relocation DMA issued once per group after both pairs' casts.
        # fb sits 3 iterations behind fa so the qmove DMA latency (~2.5us)
        # is fully hidden; back sits 2 behind fb.
        for p in range(npair + 5):
            if p >= 5:
                back(p - 5)
            if p < npair:
                front_a(p)
            if 3 <= p < npair + 3:
                front_b(p - 3)
            if p % 2 == 1 and p < npair:
                qmove(p // 2)

    nc.finalize()
    return nc


def _host_inputs(x, Wq, Wk, Wv):
    B_, T_, C_ = x.shape
    assert (B_, T_, C_) == (B, T, C), (B_, T_, C_)
    # xt4[g, p, c, j] = x[4g + j//256, j%256, 128c + p], bf16
    xh = np.ascontiguousarray(
        x.reshape(B // 4, 4, T, NCHUNK, 128).transpose(0, 4, 3, 1, 2)
        .reshape(B // 4, 128, NCHUNK, 4 * T)
    ).astype(BF)
    wkq_h = np.ascontiguousarray(
        np.concatenate([Wk, Wq], axis=1).reshape(NCHUNK, 128, 128).transpose(1, 0, 2)
    ).astype(BF)
    wv_h = np.ascontiguousarray(
        Wv.reshape(NCHUNK, 128, H).transpose(1, 0, 2)
    ).astype(BF)
    mask_h = np.triu(np.ones((128, 128), dtype=np.float32)).astype(BF)
    return xh, wkq_h, wv_h, mask_h


def _host_output(res, bpc=BPC):
    # outF [ngroup, 128, 8, 65] bf16: block 2j+k = batch j of group, t-half k;
    # col 64 = softmax denominator. Divide on host in fp32.
    outs = []
    for i in range(N_CORES):
        a = np.asarray(res.results[i]["outF"]).astype(np.float32)
        a = a.reshape(bpc // 4, 128, 4, 2, H + 1).transpose(0, 2, 3, 1, 4)
        a = a.reshape(bpc, T, H + 1)
        outs.append(a[:, :, 0:H] / a[:, :, H : H + 1])
    return np.ascontiguousarray(np.concatenate(outs, axis=0))


def kernel(x, Wq, Wk, Wv):
    x = np.asarray(x, dtype=np.float32)
    Wq = np.asarray(Wq, dtype=np.float32)
    Wk = np.asarray(Wk, dtype=np.float32)
    Wv = np.asarray(Wv, dtype=np.float32)

    xh, wkq_h, wv_h, mask_h = _host_inputs(x, Wq, Wk, Wv)

    nc = build_nc(BPC)
    in_maps = [
        {
            "xt4": xh[i * NGROUP : (i + 1) * NGROUP],
            "wkq": wkq_h,
            "wv": wv_h,
            "mask": mask_h,
        }
        for i in range(N_CORES)
    ]
    res = run_bass_kernel_spmd(nc, in_maps, list(range(N_CORES)))
    return _host_output(res, BPC)


# revision 29
# speedup vs baseline: 1.0130x; 1.0130x over previous
"""Single-head causal self-attention on 8 Trainium2 NeuronCores (Bass/Tile).

Problem: x [1024, 256, 384], Wq/Wk/Wv [384, 64] ->
  q,k,v = x@W;  wei = softmax(mask(q k^T / sqrt(384)));  out = wei @ v
Output: [1024, 256, 64] fp32.

Strategy (data-parallel over batch, 128 batches per core):
  - Host pre-transposes x to bf16 xt4[g, p, c, j] = x[4g + j//256, j%256,
    128c+p] (groups of 4 batches = 2 pairs): contraction dim C=384 on SBUF
    partitions, one 6KB-per-partition DMA per group.
  - Per batch pair:
      ps_qk [128,512] = [Wk|Wq]^T x^T          (3 mm, N=512, bf16)
      kq slot [128,640] fp8e4 <- ONE DVE cast from PSUM (k rows 0:64,
        q rows 64:128); cols 512:640 hold persistent zeros.
      v-proj reuses x-stationary form (12 mm, N=64, bf16) -> ps_v [t,h];
        one gpsimd copy -> vaug [128,2,2,66] bf16 w/ ones at col 64.
      psw [128,1024] = wei^T blocks via fp8 DoubleRow matmuls: the second
        k-tile is the zero pad (cols 512:640), so every wei matmul runs at
        0.5 cycles/row (4 mm per pair).
      P [128,2,384] bf16 = exp(psw/sqrt(384))   (ONE ACT op per pair)
      mask diag blocks via 4D strided AP        (ONE DVE mul per pair)
      pso [128,4,65]: out[t,h]+denom: lhsT=P-block stationary, rhs=vaug
        (3 mm per batch, N=65, bf16; col 64 = softmax denominator)
      out_sb bf16 <- gpsimd copy of pso (numerator + denominator shipped;
        the divide happens on host in fp32).
  - Three-stage software pipeline (front_a p | front_b p-1 | back p-2).
  - Output [g, p, 8, 65] bf16 blocks; host divides and reassembles [b,t,h].
"""

from contextlib import ExitStack

import numpy as np
import ml_dtypes

import concourse.bass as bass
import concourse.bacc as bacc
import concourse.tile as tile
from concourse import mybir
from concourse.bass_utils import run_bass_kernel_spmd

N_CORES = 8
B = 1024
T = 256
C = 384
H = 64
BPC = B // N_CORES  # 128 batches per core
NCHUNK = C // 128  # 3
NGROUP = BPC // 4  # 32 groups (2 pairs) per core
SCALE = float(C) ** -0.5

F32 = mybir.dt.float32
BF16 = mybir.dt.bfloat16
F8 = mybir.dt.float8e4
BF = ml_dtypes.bfloat16


def build_nc(bpc: int = BPC):
    npair = bpc // 2
    ngroup = bpc // 4
    nc = bacc.Bacc(
        "TRN2", target_bir_lowering=False, debug=False, num_devices=N_CORES
    )

    xt4 = nc.dram_tensor("xt4", [ngroup, 128, NCHUNK, 1024], BF16, kind="ExternalInput").ap()
    wkq = nc.dram_tensor("wkq", [128, NCHUNK, 128], BF16, kind="ExternalInput").ap()
    wv = nc.dram_tensor("wv", [128, NCHUNK, H], BF16, kind="ExternalInput").ap()
    mask = nc.dram_tensor("mask", [128, 128], BF16, kind="ExternalInput").ap()
    outF = nc.dram_tensor("outF", [ngroup, 128, 8, H + 1], BF16, kind="ExternalOutput").ap()

    with ExitStack() as ctx:
        tc = ctx.enter_context(tile.TileContext(nc))

        const = ctx.enter_context(tc.tile_pool(name="const", bufs=1))
        xt_pool = ctx.enter_context(tc.tile_pool(name="xt", bufs=3))
        wkq_sb = const.tile([128, NCHUNK, 128], BF16, tag="wkq")
        nc.sync.dma_start(wkq_sb[:], wkq)
        wv_sb = const.tile([128, NCHUNK, H], BF16, tag="wv")
        nc.sync.dma_start(wv_sb[:], wv)
        # Cached fill register for the gpsimd affine-select causal mask.
        fill_reg = nc.gpsimd.to_reg(0.0)

        NSLOT = 4
        # Persistent kq slots [128, slot, 5, 128] fp8: k^T rows 0:64 /
        # q^T rows 64:128 in blocks 0:4; block 4 stays zero (the DoubleRow
        # zero k-tile). One tile so the per-group q-relocation DMA can cover
        # two adjacent slots with a single strided access pattern.
        kqs = const.tile([128, NSLOT, 5, 128], F8, tag="kqs")
        nc.gpsimd.memset(kqs[:, :, 4, :], 0.0)
        # q relocated to base-partition 0 (matmul fmap must share the weight
        # tile's start partition): qs[0:64, slot, t] <- kqs[64:128, slot, t].
        qs = const.tile([64, NSLOT, 512], F8, tag="qs")
        # Persistent v_aug pair-tiles [batch, s-half, 66]: v at [., j, i, 0:64],
        # ones at col 64 (softmax denominator trick). 6 slots span the
        # 4-stage pipeline distance between the v copy and the PV matmuls.
        NSLOT_V = 6
        vaug = []
        for i in range(NSLOT_V):
            v_t = const.tile([128, 2, 2, 66], BF16, tag=f"vaug{i}")
            nc.gpsimd.memset(v_t[:, :, :, 64:65], 1.0)
            vaug.append(v_t)

        p_pool = ctx.enter_context(tc.tile_pool(name="pp", bufs=5))
        o_pool = ctx.enter_context(tc.tile_pool(name="oo", bufs=2))
        psqk_pool = ctx.enter_context(tc.tile_pool(name="psqk", bufs=2, space="PSUM"))
        psv_pool = ctx.enter_context(tc.tile_pool(name="psv", bufs=2, space="PSUM"))
        psw_pool = ctx.enter_context(tc.tile_pool(name="psw", bufs=3, space="PSUM"))
        pso_pool = ctx.enter_context(tc.tile_pool(name="pso", bufs=1, space="PSUM"))

        xt_tiles = {}
        st_a = {}  # pair -> fa state
        st_b = {}  # pair -> (P, vaug slot)
        st_p = {}  # pair -> pso
        st_o = {}  # group -> out_sb

        def front_a(p):
            """DMA + qk projection + fp8 cast + v projection for pair p."""
            g, r = divmod(p, 2)
            if r == 0:
                xt = xt_pool.tile([128, NCHUNK, 1024], BF16, tag="xt")
                nc.sync.dma_start(xt[:], xt4[g])
                xt_tiles[g] = xt
            xt = xt_tiles[g]
            base = 512 * r

            ps_qk = psqk_pool.tile([128, 512], F32, tag="psqk")
            for c in range(NCHUNK):
                nc.tensor.matmul(
                    ps_qk[:],
                    lhsT=wkq_sb[:, c, :],
                    rhs=xt[:, c, base : base + 512],
                    start=(c == 0),
                    stop=(c == NCHUNK - 1),
                )
            s = p % NSLOT
            nc.vector.tensor_copy(
                kqs[:, s, 0:4, :].rearrange("p a b -> p (a b)"), ps_qk[:]
            )

            ps_v = psv_pool.tile([128, 4, H], F32, tag="psv")
            for blk in range(4):
                for c in range(NCHUNK):
                    nc.tensor.matmul(
                        ps_v[:, blk, :],
                        lhsT=xt[:, c, base + 128 * blk : base + 128 * (blk + 1)],
                        rhs=wv_sb[:, c, :],
                        start=(c == 0),
                        stop=(c == NCHUNK - 1),
                    )
            st_a[p] = (s, vaug[p % NSLOT_V], ps_v)

        def fa_vcopy(p):
            """v PSUM->SBUF copies, emitted late so the ACT/DVE queues put
            the critical exp / out-copy work first."""
            s, vslot, ps_v = st_a[p]
            nc.scalar.copy(vslot[:, 0, :, 0:64], ps_v[:, 0:2, :])
            nc.vector.tensor_copy(vslot[:, 1, :, 0:64], ps_v[:, 2:4, :])
            st_a[p] = (s, vslot)

        def qmove(g):
            """Relocate both pairs' q halves to partition base 0 (one DMA)."""
            sA = (2 * g) % NSLOT
            nc.sync.dma_start(
                qs[:, sA : sA + 2, :],
                kqs[64:128, sA : sA + 2, 0:4, :].rearrange("p a b c -> p a (b c)"),
            )

        def front_b(p):
            """wei DoubleRow matmuls + exp + mask for pair p, per batch so
            the wei->exp->wei loop-carried chain interleaves across the two
            one-bank psw buffers instead of serializing pair-to-pair."""
            s, vslot = st_a.pop(p)

            P = p_pool.tile([128, 2, 3, 128], BF16, tag="p")
            for j in range(2):
                psw = psw_pool.tile([128, 512], F32, tag="psw")
                # fp8 DoubleRow: k-tile 0 is the real k block (block i of the
                # slot), k-tile 1 is the persistent zero pad at block index 4
                # -- step-slice i::(4-i) selects blocks {i, 4}.
                # (s-block 0) x (t 0:256) -> cols 0:256
                nc.tensor.matmul(
                    psw[:, 0:256],
                    lhsT=kqs[0:64, s, 2 * j :: 4 - 2 * j, :],
                    rhs=qs[:, s, 256 * j : 256 * j + 256]
                    .unsqueeze(1)
                    .broadcast_to([64, 2, 256]),
                    start=True,
                    stop=True,
                    perf_mode=mybir.MatmulPerfMode.DoubleRow,
                )
                # (s-block 1) x (t 128:256) -> cols 256:384
                nc.tensor.matmul(
                    psw[:, 256:384],
                    lhsT=kqs[0:64, s, 2 * j + 1 :: 3 - 2 * j, :],
                    rhs=qs[:, s, 256 * j + 128 : 256 * j + 256]
                    .unsqueeze(1)
                    .broadcast_to([64, 2, 128]),
                    start=True,
                    stop=True,
                    perf_mode=mybir.MatmulPerfMode.DoubleRow,
                )
                # Blocks land as [s0t0 | s0t1 | s1t1] = P[:, j, {0,1,2}, :].
                nc.scalar.activation(
                    P[:, j, :, :].rearrange("p a b -> p (a b)"),
                    psw[:, 0:384],
                    mybir.ActivationFunctionType.Exp,
                    scale=SCALE,
                )
                # Causal mask on the diag blocks (0 and 2) via GPSIMD affine
                # select: keep P[s, t] where t - s >= 0, else fill 0.
                nc.gpsimd.affine_select(
                    out=P[:, j, 0::2, :],
                    in_=P[:, j, 0::2, :],
                    pattern=[[0, 2], [1, 128]],
                    compare_op=mybir.AluOpType.is_ge,
                    fill=fill_reg,
                    base=0,
                    channel_multiplier=-1,
                )
            st_b[p] = (P, vslot)

        def back_mm(p):
            """Out matmuls for pair p (PE, emitted last in the iteration)."""
            P, vslot = st_b.pop(p)
            pso_full = pso_pool.tile([128, 4, 2 * H], F32, tag="pso")
            pso = pso_full[:, :, 0 : H + 1]
            for j in range(2):
                nc.tensor.matmul(
                    pso[:, 2 * j, :],
                    lhsT=P[:, j, 0, :],
                    rhs=vslot[:, j, 0, 0:65],
                    start=True,
                    stop=True,
                )
                nc.tensor.matmul(
                    pso[:, 2 * j + 1, :],
                    lhsT=P[:, j, 1, :],
                    rhs=vslot[:, j, 0, 0:65],
                    start=True,
                    stop=False,
                )
                nc.tensor.matmul(
                    pso[:, 2 * j + 1, :],
                    lhsT=P[:, j, 2, :],
                    rhs=vslot[:, j, 1, 0:65],
                    start=False,
                    stop=True,
                )
            st_p[p] = pso

        def back_copy(p):
            """Output copy (head of the DVE queue: it releases the single
            pso bank for the next PV) + output DMA."""
            g, r = divmod(p, 2)
            pso = st_p.pop(p)
            if r == 0:
                out_sb = o_pool.tile([128, 8, H + 1], BF16, tag="out")
                st_o[g] = out_sb
            else:
                out_sb = st_o[g]
            nc.vector.tensor_copy(out_sb[:, 4 * r : 4 * r + 4, :], pso)
            if r == 1:
                nc.sync.dma_start(outF[g], st_o.pop(g)[:])

        # Software pipeline (emission order tuned so each engine's in-order
        # queue sees its critical op first): out-copy(it-5) heads the DVE
        # queue, exps(it-3) head ACT before the v copies(it), PV(it-4) runs
        # last on the PE. fb sits 3 iterations behind fa so the qmove DMA
        # latency (~2.5us) is fully hidden.
        for it in range(npair + 5):
            if it >= 5:
                back_copy(it - 5)
            if it < npair:
                front_a(it)
            if 3 <= it < npair + 3:
                front_b(it - 3)
            if it < npair:
                fa_vcopy(it)
            if 4 <= it < npair + 4:
                back_mm(it - 4)
            if it % 2 == 1 and it < npair:
                qmove(it // 2)

    nc.finalize()
    return nc


def _host_inputs(x, Wq, Wk, Wv):
    B_, T_, C_ = x.shape
    assert (B_, T_, C_) == (B, T, C), (B_, T_, C_)
    # xt4[g, p, c, j] = x[4g + j//256, j%256, 128c + p], bf16
    xh = np.ascontiguousarray(
        x.reshape(B // 4, 4, T, NCHUNK, 128).transpose(0, 4, 3, 1, 2)
        .reshape(B // 4, 128, NCHUNK, 4 * T)
    ).astype(BF)
    wkq_h = np.ascontiguousarray(
        np.concatenate([Wk, Wq], axis=1).reshape(NCHUNK, 128, 128).transpose(1, 0, 2)
    ).astype(BF)
    wv_h = np.ascontiguousarray(
        Wv.reshape(NCHUNK, 128, H).transpose(1, 0, 2)
    ).astype(BF)
    mask_h = np.triu(np.ones((128, 128), dtype=np.float32)).astype(BF)
    return xh, wkq_h, wv_h, mask_h


def _host_output(res, bpc=BPC):
    # outF [ngroup, 128, 8, 65] bf16: block 2j+k = batch j of group, t-half k;
    # col 64 = softmax denominator. Divide on host in fp32.
    outs = []
    for i in range(N_CORES):
        a = np.asarray(res.results[i]["outF"]).astype(np.float32)
        a = a.reshape(bpc // 4, 128, 4, 2, H + 1).transpose(0, 2, 3, 1, 4)
        a = a.reshape(bpc, T, H + 1)
        outs.append(a[:, :, 0:H] / a[:, :, H : H + 1])
    return np.ascontiguousarray(np.concatenate(outs, axis=0))


def kernel(x, Wq, Wk, Wv):
    x = np.asarray(x, dtype=np.float32)
    Wq = np.asarray(Wq, dtype=np.float32)
    Wk = np.asarray(Wk, dtype=np.float32)
    Wv = np.asarray(Wv, dtype=np.float32)

    xh, wkq_h, wv_h, mask_h = _host_inputs(x, Wq, Wk, Wv)

    nc = build_nc(BPC)
    in_maps = [
        {
            "xt4": xh[i * NGROUP : (i + 1) * NGROUP],
            "wkq": wkq_h,
            "wv": wv_h,
            "mask": mask_h,
        }
        for i in range(N_CORES)
    ]
    res = run_bass_kernel_spmd(nc, in_maps, list(range(N_CORES)))
    return _host_output(res, BPC)
